# revision 28
# baseline (speedup 1.0000x reference)
"""GQA causal attention block (q/k/v proj + softmax attention + out proj),
tensor-parallel over 8 NeuronCores.

Reference semantics (fp32):
  q = x @ Wq  -> [s, 32, 64];  k,v = x @ Wk/Wv -> [s, 8, 64]
  GQA repeat kv x4, causal softmax(q k^T / 8) @ v, concat -> @ Wo + bo

Sharding: core i owns q-heads 4i..4i+3 and kv-head i (kv groups aligned),
Wo rows 256i..256i+256.  Each core computes a full-shape partial output;
host sums the 8 partials and adds bo.

v2 design notes:
  qt [64, 4*512] f16: all 4 heads on partitions 0:64, side by side in the
  free dim, so one kT block serves all heads as matmul stationary.
  Scores per (pair, block): wide matmuls into a pair psum tile
  [128, 1024] f32 (head h at cols 512h'); exp'd pair-wide on the Act
  engine.  Diagonal blocks compute/exp only columns right of the
  diagonal; masking = memset of fully-masked cols + a 128-wide triangle
  affine_select on gpsimd.
  ctx accumulates v.T @ ex into a pair psum tile [65, 1024] across all
  blocks; v carries a ones column so row 64 is the softmax denominator.
  Normalize: reciprocal_approx_fast on the denom row (DVE), gpsimd
  partition_broadcast, DVE multiply, DMA to pack the odd head into
  partitions 64:128 for the out projection.
  proj(c+1) / oproj(c-1) matmuls are interleaved into the attention
  block loops as fillers so the PE never waits on the Act engine.
"""

from collections import deque

import numpy as np

HEAD_DIM = 64
N_CORES = 8
S = 4096
D_IN = 2048
D_OUT = 2048
BLK = 128
CH = 512  # sq chunk width
SCALE = 1.0 / 8.0  # 1/sqrt(64)
WIDE = False  # N=1024 matmul outputs fail walrus s3d3_mm_num_elements (psum bank)
TRIM = True  # trim diag-block scores/exp to columns right of the diagonal
FILL = True  # interleave proj/oproj pieces into attention passes
DEBUG_DUMP = False  # dump chunk-0 intermediates to DRAM for HW debugging

_NC_CACHE = {}


def _enable_ldw_opt():
    # walrus ships with --enable-ldw-opt=false; the LDW optimization pass
    # merges/elides redundant LDWEIGHTS, which this kernel's
    # shared-stationary matmul ordering benefits from.
    from concourse import bass_utils as _bu

    if getattr(_bu, "_ldw_patched", False):
        return
    _orig = _bu.run_command

    def patched(argv, **kw):
        argv = [
            "--enable-ldw-opt=true" if a == "--enable-ldw-opt=false" else a
            for a in argv
        ]
        return _orig(argv, **kw)

    _bu.run_command = patched
    _bu._ldw_patched = True


def _build(s):
    from contextlib import ExitStack

    import concourse.mybir as mybir
    import concourse.tile as tile
    from concourse import bacc
    from concourse.bass import ts
    from concourse.masks import make_identity

    from concourse import library_config

    f32 = mybir.dt.float32
    f16 = mybir.dt.float16
    Exp = mybir.ActivationFunctionType.Exp
    nch = s // CH

    nc = bacc.Bacc("TRN2", target_bir_lowering=False, debug=False)
    xT = nc.dram_tensor("xT", [D_IN, s], f16, kind="ExternalInput")
    wq = nc.dram_tensor("wq", [2, 128, 16, 128], f16, kind="ExternalInput")
    wkv = nc.dram_tensor("wkv", [128, 16, 128], f16, kind="ExternalInput")
    wo = nc.dram_tensor("wo", [2, 128, D_OUT], f16, kind="ExternalInput")
    out = nc.dram_tensor("out", [s, D_OUT], f16, kind="ExternalOutput")
    if DEBUG_DUMP:
        dbg = {
            "dbg_qt": nc.dram_tensor("dbg_qt", [128, 4 * CH], f16, kind="ExternalOutput"),
            "dbg_kt": nc.dram_tensor("dbg_kt", [128, CH], f16, kind="ExternalOutput"),
            "dbg_vt": nc.dram_tensor("dbg_vt", [128, CH], f16, kind="ExternalOutput"),
            "dbg_vn": nc.dram_tensor("dbg_vn", [128, 4 * (HEAD_DIM + 1)], f16, kind="ExternalOutput"),
            "dbg_cq0": nc.dram_tensor("dbg_cq0", [128, CH], f16, kind="ExternalOutput"),
            "dbg_cq1": nc.dram_tensor("dbg_cq1", [128, CH], f16, kind="ExternalOutput"),
            "dbg_rb": nc.dram_tensor("dbg_rb", [128, 2 * CH], f32, kind="ExternalOutput"),
            "dbg_rs": nc.dram_tensor("dbg_rs", [128, 2 * CH], f32, kind="ExternalOutput"),
            "dbg_ex": nc.dram_tensor("dbg_ex", [128, 2 * CH], f16, kind="ExternalOutput"),
            "dbg_sc": nc.dram_tensor("dbg_sc", [128, 2 * CH], f32, kind="ExternalOutput"),
        }

    with tile.TileContext(nc) as tc, ExitStack() as ctx:
        singles = ctx.enter_context(tc.tile_pool(name="singles", bufs=1))
        wq_sb = [singles.tile([128, 16, 128], f16, tag=f"wq{p}", name=f"wq{p}") for p in range(2)]
        wkv_sb = singles.tile([128, 16, 128], f16, tag="wkv", name="wkv")
        wo_sb = [singles.tile([128, D_OUT], f16, tag=f"wo{p}", name=f"wo{p}") for p in range(2)]
        ident = singles.tile([128, 128], f16, tag="ident", name="ident")

        # partition_broadcast needs the attn gpsimd ucode library (the
        # default `standard` library silently no-ops it on hardware)
        nc.gpsimd.load_library(library_config.attn)

        make_identity(nc, ident)

        # per-chunk static tiles (bufs = nch, requested once per chunk)
        kt_pool = ctx.enter_context(tc.tile_pool(name="kt", bufs=nch))
        vt_pool = ctx.enter_context(tc.tile_pool(name="vt", bufs=nch))
        v_pool = ctx.enter_context(tc.tile_pool(name="vn", bufs=nch))
        # rotating work tiles
        xt_pool = ctx.enter_context(tc.tile_pool(name="xt", bufs=2))
        qt_pool = ctx.enter_context(tc.tile_pool(name="qt", bufs=2))
        qs_pool = ctx.enter_context(tc.tile_pool(name="qs", bufs=3))
        ex_pool = ctx.enter_context(tc.tile_pool(name="ex", bufs=3))
        cq_pool = ctx.enter_context(tc.tile_pool(name="cq", bufs=4))
        tmp_pool = ctx.enter_context(tc.tile_pool(name="tmp", bufs=2))
        rs_pool = ctx.enter_context(tc.tile_pool(name="rs", bufs=2))
        rb_pool = ctx.enter_context(tc.tile_pool(name="rb", bufs=2))
        cs_pool = ctx.enter_context(tc.tile_pool(name="cs", bufs=2))
        osb_pool = ctx.enter_context(tc.tile_pool(name="osb", bufs=2))
        ps_sc = ctx.enter_context(tc.tile_pool(name="pssc", bufs=2, space="PSUM"))
        ps_ctx = ctx.enter_context(tc.tile_pool(name="psctx", bufs=1, space="PSUM"))
        ps_mix = ctx.enter_context(tc.tile_pool(name="psmix", bufs=2, space="PSUM"))

        xts, kts, vts, vns, qts, cqs = {}, {}, {}, {}, {}, {}
        dbg_rb_tile = []
        if DEBUG_DUMP:
            dbg_ex_sb = singles.tile([128, 2 * CH], f16, tag="dbgex", name="dbgex")
            dbg_sc_sb = singles.tile([128, 2 * CH], f32, tag="dbgsc", name="dbgsc")

        def dma_xt(c):
            t = xt_pool.tile([128, 16, CH], f16, tag="xt", name="xt")
            for h in range(2):
                nc.sync.dma_start(
                    out=t[:, ts(h, 8), :],
                    in_=xT[ts(h, 1024), ts(c, CH)].rearrange(
                        "(kc p) n -> p kc n", p=128
                    ),
                )
            xts[c] = t

        def proj_pieces(c):
            """q/kv projections + v transpose for chunk c -> list of thunks."""
            pieces = []
            st = {}

            def q_mms(p, k0):
                def f():
                    if k0 == 0:
                        st[f"pq{p}"] = ps_mix.tile([128, CH], f32, tag="mix", name="pq")
                    pq = st[f"pq{p}"]
                    for k in range(k0, k0 + 4):
                        nc.tensor.matmul(
                            pq, wq_sb[p][:, k, :], xts[c][:, k, :],
                            start=(k == 0), stop=(k == 15),
                        )
                return f

            def q_fix(p):
                def f():
                    pq = st[f"pq{p}"]
                    if p == 0:
                        qts[c] = qt_pool.tile([128, 4 * CH], f16, tag="qt", name="qt")
                    qt_t = qts[c]
                    nc.vector.tensor_copy(
                        out=qt_t[0:64, ts(2 * p, CH)], in_=pq[0:64, :]
                    )
                    qs_t = qs_pool.tile([128, CH], f16, tag="qs", name="qs")
                    nc.vector.tensor_copy(out=qs_t[64:128, :], in_=pq[64:128, :])
                    nc.sync.dma_start(
                        out=qt_t[0:64, ts(2 * p + 1, CH)], in_=qs_t[64:128, :]
                    )
                return f

            def kv_mms(k0):
                def f():
                    if k0 == 0:
                        st["pkv"] = ps_mix.tile([128, CH], f32, tag="mix", name="pkv")
                    pkv = st["pkv"]
                    for k in range(k0, k0 + 4):
                        nc.tensor.matmul(
                            pkv, wkv_sb[:, k, :], xts[c][:, k, :],
                            start=(k == 0), stop=(k == 15),
                        )
                return f

            def kv_fix():
                # psum rows 0:64 = vT chunk, rows 64:128 = kT chunk
                pkv = st["pkv"]
                vts[c] = vt_pool.tile([128, CH], f16, tag="vt", name="vt")
                nc.vector.tensor_copy(out=vts[c][0:64, :], in_=pkv[0:64, :])
                ks_t = qs_pool.tile([128, CH], f16, tag="qs", name="ks")
                nc.vector.tensor_copy(out=ks_t[64:128, :], in_=pkv[64:128, :])
                kts[c] = kt_pool.tile([128, CH], f16, tag="kt", name="kt")
                nc.sync.dma_start(out=kts[c][0:64, :], in_=ks_t[64:128, :])
                vns[c] = v_pool.tile([128, 4, HEAD_DIM + 1], f16, tag="vn", name="vn")
                nc.vector.memset(vns[c][:, :, HEAD_DIM : HEAD_DIM + 1], 1.0)

            def v_tr(j):
                def f():
                    pv = ps_mix.tile([128, CH], f16, tag="mix", name="pv")
                    nc.tensor.transpose(
                        pv[:, 0:HEAD_DIM],
                        vts[c][0:64, ts(j, BLK)],
                        ident[0:64, 0:64],
                    )
                    nc.vector.tensor_copy(
                        out=vns[c][:, j, 0:HEAD_DIM], in_=pv[:, 0:HEAD_DIM]
                    )
                return f

            # first-half k-steps (k<8) for both pairs precede second halves,
            # so the prologue overlaps the second xt DMA half with matmuls
            pieces.append(q_mms(0, 0))
            pieces.append(q_mms(0, 4))
            pieces.append(q_mms(1, 0))
            pieces.append(q_mms(1, 4))
            pieces.append(q_mms(0, 8))
            pieces.append(q_mms(0, 12))
            pieces.append(q_fix(0))
            pieces.append(q_mms(1, 8))
            pieces.append(q_mms(1, 12))
            pieces.append(q_fix(1))
            for k0 in (0, 4, 8, 12):
                pieces.append(kv_mms(k0))
            pieces.append(kv_fix)
            for j in range(4):
                pieces.append(v_tr(j))
            return pieces

        def oproj_pieces(c):
            """out projection for chunk c (after normalize(c)) -> thunks."""
            pieces = []
            st = {}

            def mm(jj, g):
                def f():
                    if g == 0:
                        st[f"osb{jj}"] = osb_pool.tile(
                            [128, D_OUT], f16, tag="osb", name="osb"
                        )
                    po = [
                        ps_mix.tile([128, CH], f32, tag="mix", name="po")
                        for _ in range(2)
                    ]
                    for p in range(2):
                        for n in range(2):
                            nc.tensor.matmul(
                                po[n],
                                cqs[c][p][:, ts(jj, 128)],
                                wo_sb[p][:, ts(2 * g + n, CH)],
                                start=(p == 0), stop=(p == 1),
                            )
                    for n in range(2):
                        nc.vector.tensor_copy(
                            out=st[f"osb{jj}"][:, ts(2 * g + n, CH)], in_=po[n]
                        )
                return f

            def dma(jj):
                def f():
                    nc.sync.dma_start(
                        out=out[ts(4 * c + jj, BLK), :], in_=st[f"osb{jj}"]
                    )
                return f

            for jj in range(4):
                pieces.append(mm(jj, 0))
                pieces.append(mm(jj, 1))
                pieces.append(dma(jj))
            return pieces

        def do_pass(c, p, fillers):
            nblk = 4 * c + 4
            total = len(fillers)
            qt_t = qts[c]
            pctx = ps_ctx.tile([128, 2 * CH], f32, tag="ctx", name="pctx")
            # diag-first order (c>=1): the full-width jd=0 ctx matmul opens
            # the accumulation for every column, later diag blocks accumulate
            # only the columns right of the diagonal, and the last non-diag
            # block closes the group full-width.
            if TRIM and c >= 1:
                order = list(range(4 * c, nblk)) + list(range(4 * c))
            else:
                order = list(range(nblk))
            for bi, b in enumerate(order):
                jd = b - 4 * c
                cc, j = b // 4, b % 4
                kt = kts[cc]
                sc = ps_sc.tile([128, 2 * CH], f32, tag="sc", name="sc")
                if jd < 0 and WIDE:
                    nc.tensor.matmul(
                        sc, kt[0:64, ts(j, BLK)], qt_t[0:64, ts(p, 2 * CH)],
                        start=True, stop=True,
                    )
                elif jd < 0:
                    for hh in range(2):
                        nc.tensor.matmul(
                            sc[:, ts(hh, CH)],
                            kt[0:64, ts(j, BLK)],
                            qt_t[0:64, ts(2 * p + hh, CH)],
                            start=True, stop=True,
                        )
                elif TRIM:
                    col0 = BLK * jd
                    for hh in range(2):
                        nc.tensor.matmul(
                            sc[:, hh * CH + col0 : (hh + 1) * CH],
                            kt[0:64, ts(j, BLK)],
                            qt_t[0:64, (2 * p + hh) * CH + col0 : (2 * p + hh + 1) * CH],
                            start=True, stop=True,
                        )
                else:
                    for hh in range(2):
                        nc.tensor.matmul(
                            sc[:, ts(hh, CH)],
                            kt[0:64, ts(j, BLK)],
                            qt_t[0:64, ts(2 * p + hh, CH)],
                            start=True, stop=True,
                        )
                ex = ex_pool.tile([128, 2 * CH], f16, tag="ex", name="ex")
                if jd < 0:
                    nc.scalar.activation(out=ex, in_=sc, func=Exp, scale=SCALE)
                elif TRIM:
                    col0 = BLK * jd
                    for hh in range(2):
                        nc.scalar.activation(
                            out=ex[:, hh * CH + col0 : (hh + 1) * CH],
                            in_=sc[:, hh * CH + col0 : (hh + 1) * CH],
                            func=Exp, scale=SCALE,
                        )
                    if jd > 0 and c == 0:
                        for hh in range(2):
                            nc.vector.memset(ex[:, hh * CH : hh * CH + col0], 0.0)
                    for hh in range(2):
                        # keep where q_local - r >= 0 within the 128-wide
                        # diagonal sub-block
                        nc.gpsimd.affine_select(
                            out=ex[:, hh * CH + col0 : hh * CH + col0 + BLK],
                            in_=ex[:, hh * CH + col0 : hh * CH + col0 + BLK],
                            compare_op=mybir.AluOpType.is_ge,
                            fill=0.0,
                            base=0,
                            pattern=[[1, BLK]],
                            channel_multiplier=-1,
                        )
                else:
                    nc.scalar.activation(out=ex, in_=sc, func=Exp, scale=SCALE)
                    for hh in range(2):
                        # baseline-style full-width diagonal mask:
                        # keep where q - r - 128*jd >= 0
                        nc.gpsimd.affine_select(
                            out=ex[:, ts(hh, CH)],
                            in_=ex[:, ts(hh, CH)],
                            compare_op=mybir.AluOpType.is_ge,
                            fill=0.0,
                            base=-BLK * jd,
                            pattern=[[1, CH]],
                            channel_multiplier=-1,
                        )
                if DEBUG_DUMP and c == 0 and p == 0 and b == 0:
                    nc.vector.tensor_copy(out=dbg_ex_sb, in_=ex)
                    nc.vector.tensor_copy(out=dbg_sc_sb, in_=sc)
                st_f = bi == 0
                sp_f = bi == nblk - 1
                if TRIM and c >= 1 and jd >= 1:
                    col0 = BLK * jd
                    for hh in range(2):
                        nc.tensor.matmul(
                            pctx[: HEAD_DIM + 1, hh * CH + col0 : (hh + 1) * CH],
                            vns[cc][:, j, :],
                            ex[:, hh * CH + col0 : (hh + 1) * CH],
                            start=st_f, stop=sp_f,
                        )
                else:
                    for hh in range(2):
                        nc.tensor.matmul(
                            pctx[: HEAD_DIM + 1, ts(hh, CH)],
                            vns[cc][:, j, :], ex[:, ts(hh, CH)],
                            start=st_f, stop=sp_f,
                        )
                # drain fillers proportionally, starting from block 2
                if total and bi >= 2:
                    want = total * (bi - 1) // max(1, nblk - 2)
                    while len(fillers) > total - want:
                        fillers.popleft()()
            while fillers:
                fillers.popleft()()
            return pctx

        def normalize(c, p, pctx):
            # Stage ctx psum rows to SBUF immediately so the single psum ctx
            # tile frees fast (next pass's accumulation can start).
            # The custom DVE uop (reciprocal_approx_fast) and the gpsimd
            # ucode op (partition_broadcast) only work at partition base 0
            # on hardware, so DMA the denominator row down to partition 0.
            cs_t = cs_pool.tile([128, 2 * CH], f32, tag="cs", name="cs")
            nc.vector.tensor_copy(out=cs_t[0:65, :], in_=pctx[0:65, :])
            rs_t = rs_pool.tile([128, 2 * CH], f32, tag="rs", name="rs")
            nc.sync.dma_start(out=rs_t[0:1, :], in_=cs_t[64:65, :])
            rcp_t = rs_pool.tile([128, 2 * CH], f32, tag="rcp", name="rcp")
            nc.vector.reciprocal_approx_fast(
                out=rcp_t[0:1, :], in_=rs_t[0:1, :]
            )
            rb_t = rb_pool.tile([128, 2 * CH], f32, tag="rb", name="rb")
            nc.gpsimd.partition_broadcast(rb_t[0:64, :], rcp_t[0:1, :], channels=64)
            cq_t = cq_pool.tile([128, CH], f16, tag="cq", name="cq")
            tm_t = tmp_pool.tile([128, CH], f16, tag="tmp", name="tmp")
            nc.vector.tensor_mul(tm_t[0:64, :], cs_t[0:64, ts(1, CH)], rb_t[0:64, ts(1, CH)])
            nc.sync.dma_start(out=cq_t[64:128, :], in_=tm_t[0:64, :])
            nc.vector.tensor_mul(cq_t[0:64, :], cs_t[0:64, ts(0, CH)], rb_t[0:64, ts(0, CH)])
            cqs.setdefault(c, {})[p] = cq_t
            if DEBUG_DUMP and c == 0 and p == 0:
                dbg_rb_tile.append(rb_t)
                dbg_rb_tile.append(rs_t)

        # prologue: interleave weight and x-chunk loads so the first
        # projection matmuls (needing only wq0 + the first xT half) start
        # as early as possible; wo is not needed until oproj(0)
        xt0 = xt_pool.tile([128, 16, CH], f16, tag="xt", name="xt")
        xts[0] = xt0
        nc.sync.dma_start(out=wq_sb[0], in_=wq[0, :, :, :])
        nc.sync.dma_start(
            out=xt0[:, ts(0, 8), :],
            in_=xT[ts(0, 1024), ts(0, CH)].rearrange("(kc p) n -> p kc n", p=128),
        )
        nc.sync.dma_start(out=wq_sb[1], in_=wq[1, :, :, :])
        nc.sync.dma_start(
            out=xt0[:, ts(1, 8), :],
            in_=xT[ts(1, 1024), ts(0, CH)].rearrange("(kc p) n -> p kc n", p=128),
        )
        nc.sync.dma_start(out=wkv_sb, in_=wkv[:, :, :])
        if nch > 1:
            dma_xt(1)
        for p in range(2):
            nc.scalar.dma_start(out=wo_sb[p], in_=wo[p, :, :])
        for pc in proj_pieces(0):
            pc()

        for c in range(nch):
            if c >= 1 and c + 1 < nch:
                dma_xt(c + 1)
            f0 = deque(oproj_pieces(c - 1)) if c >= 1 else deque()
            if not FILL:
                while f0:
                    f0.popleft()()
            pctx0 = do_pass(c, 0, f0)
            normalize(c, 0, pctx0)
            f1 = deque(proj_pieces(c + 1)) if c + 1 < nch else deque()
            if not FILL:
                while f1:
                    f1.popleft()()
            pctx1 = do_pass(c, 1, f1)
            normalize(c, 1, pctx1)
        for pc in oproj_pieces(nch - 1):
            pc()
        if DEBUG_DUMP:
            nc.sync.dma_start(out=dbg["dbg_qt"][:, :], in_=qts[0])
            nc.sync.dma_start(out=dbg["dbg_kt"][:, :], in_=kts[0])
            nc.sync.dma_start(out=dbg["dbg_vt"][:, :], in_=vts[0])
            nc.sync.dma_start(out=dbg["dbg_vn"][:, :], in_=vns[0].rearrange("p a b -> p (a b)"))
            nc.sync.dma_start(out=dbg["dbg_cq0"][:, :], in_=cqs[0][0])
            nc.sync.dma_start(out=dbg["dbg_cq1"][:, :], in_=cqs[0][1])
            nc.sync.dma_start(out=dbg["dbg_rb"][:, :], in_=dbg_rb_tile[0])
            nc.sync.dma_start(out=dbg["dbg_rs"][:, :], in_=dbg_rb_tile[1])
            nc.sync.dma_start(out=dbg["dbg_ex"][:, :], in_=dbg_ex_sb)
            nc.sync.dma_start(out=dbg["dbg_sc"][:, :], in_=dbg_sc_sb)

    nc.compile()
    return nc


def _get_nc(s):
    if s not in _NC_CACHE:
        _NC_CACHE[s] = _build(s)
    return _NC_CACHE[s]


def _in_maps(x, Wq, Wk, Wv, Wo, s):
    xT = np.ascontiguousarray(np.asarray(x, np.float32)[0].T.astype(np.float16))
    Wq = np.asarray(Wq, np.float32).astype(np.float16)
    Wk = np.asarray(Wk, np.float32).astype(np.float16)
    Wv = np.asarray(Wv, np.float32).astype(np.float16)
    Wo = np.asarray(Wo, np.float32).astype(np.float16)
    maps = []
    for i in range(N_CORES):
        wq_i = np.ascontiguousarray(
            Wq[:, i * 256 : (i + 1) * 256]
            .reshape(16, 128, 2, 128)
            .transpose(2, 1, 0, 3)
        )
        wkv_i = np.ascontiguousarray(
            np.concatenate(
                [Wv[:, i * 64 : (i + 1) * 64], Wk[:, i * 64 : (i + 1) * 64]], axis=1
            )
            .reshape(16, 128, 128)
            .transpose(1, 0, 2)
        )
        wo_i = np.ascontiguousarray(
            Wo[i * 256 : (i + 1) * 256, :].reshape(2, 128, D_OUT)
        )
        maps.append({"xT": xT, "wq": wq_i, "wkv": wkv_i, "wo": wo_i})
    return maps


def run(x, Wq, Wk, Wv, Wo, bo, s=S, **spmd_kwargs):
    """Builds (cached), runs on 8 cores, returns (full_output, BassKernelResults)."""
    from concourse.bass_utils import run_bass_kernel_spmd

    nc = _get_nc(s)
    maps = _in_maps(x, Wq, Wk, Wv, Wo, s)
    res = run_bass_kernel_spmd(nc, maps, core_ids=list(range(N_CORES)), **spmd_kwargs)
    acc = np.zeros((s, D_OUT), np.float64)
    for r in res.results:
        acc += r["out"].astype(np.float64)
    full = (acc + np.asarray(bo, np.float64)[None, :]).astype(np.float32)[None]
    return full, res


def kernel(x, Wq, Wk, Wv, Wo, bo):
    out, _ = run(x, Wq, Wk, Wv, Wo, bo)
    return out


# revision 30
# speedup vs baseline: 1.0586x; 1.0586x over previous
"""GQA causal attention block (q/k/v proj + softmax attention + out proj),
tensor-parallel over 8 NeuronCores.

Reference semantics (fp32):
  q = x @ Wq  -> [s, 32, 64];  k,v = x @ Wk/Wv -> [s, 8, 64]
  GQA repeat kv x4, causal softmax(q k^T / 8) @ v, concat -> @ Wo + bo

Sharding: core i owns q-heads 4i..4i+3 and kv-head i (kv groups aligned),
Wo rows 256i..256i+256.  Each core computes a full-shape partial output;
host sums the 8 partials and adds bo.

v2 design notes:
  qt [64, 4*512] f16: all 4 heads on partitions 0:64, side by side in the
  free dim, so one kT block serves all heads as matmul stationary.
  Scores per (pair, block): wide matmuls into a pair psum tile
  [128, 1024] f32 (head h at cols 512h'); exp'd pair-wide on the Act
  engine.  Diagonal blocks compute/exp only columns right of the
  diagonal; masking = memset of fully-masked cols + a 128-wide triangle
  affine_select on gpsimd.
  ctx accumulates v.T @ ex into a pair psum tile [65, 1024] across all
  blocks; v carries a ones column so row 64 is the softmax denominator.
  Normalize: reciprocal_approx_fast on the denom row (DVE), gpsimd
  partition_broadcast, DVE multiply, DMA to pack the odd head into
  partitions 64:128 for the out projection.
  proj(c+1) / oproj(c-1) matmuls are interleaved into the attention
  block loops as fillers so the PE never waits on the Act engine.
"""

from collections import deque

import numpy as np

HEAD_DIM = 64
N_CORES = 8
S = 4096
D_IN = 2048
D_OUT = 2048
BLK = 128
CH = 512  # sq chunk width
SCALE = 1.0 / 8.0  # 1/sqrt(64)
WIDE = False  # N=1024 matmul outputs fail walrus s3d3_mm_num_elements (psum bank)
TRIM = True  # trim diag-block scores/exp to columns right of the diagonal
FILL = True  # interleave proj/oproj pieces into attention passes
DEBUG_DUMP = False  # dump chunk-0 intermediates to DRAM for HW debugging

_NC_CACHE = {}


def _enable_ldw_opt():
    # walrus ships with --enable-ldw-opt=false; the LDW optimization pass
    # merges/elides redundant LDWEIGHTS, which this kernel's
    # shared-stationary matmul ordering benefits from.
    from concourse import bass_utils as _bu

    if getattr(_bu, "_ldw_patched", False):
        return
    _orig = _bu.run_command

    def patched(argv, **kw):
        argv = [
            "--enable-ldw-opt=true" if a == "--enable-ldw-opt=false" else a
            for a in argv
        ]
        return _orig(argv, **kw)

    _bu.run_command = patched
    _bu._ldw_patched = True


def _build(s):
    from contextlib import ExitStack

    import concourse.mybir as mybir
    import concourse.tile as tile
    from concourse import bacc
    from concourse.bass import ts
    from concourse.masks import make_identity

    from concourse import library_config

    f32 = mybir.dt.float32
    f16 = mybir.dt.float16
    Exp = mybir.ActivationFunctionType.Exp
    nch = s // CH

    nc = bacc.Bacc("TRN2", target_bir_lowering=False, debug=False)
    xT = nc.dram_tensor("xT", [D_IN, s], f16, kind="ExternalInput")
    wq = nc.dram_tensor("wq", [2, 128, 16, 128], f16, kind="ExternalInput")
    wkv = nc.dram_tensor("wkv", [128, 16, 128], f16, kind="ExternalInput")
    wo = nc.dram_tensor("wo", [2, 128, D_OUT], f16, kind="ExternalInput")
    out = nc.dram_tensor("out", [s, D_OUT], f16, kind="ExternalOutput")
    if DEBUG_DUMP:
        dbg = {
            "dbg_qt": nc.dram_tensor("dbg_qt", [128, 4 * CH], f16, kind="ExternalOutput"),
            "dbg_kt": nc.dram_tensor("dbg_kt", [128, CH], f16, kind="ExternalOutput"),
            "dbg_vt": nc.dram_tensor("dbg_vt", [128, CH], f16, kind="ExternalOutput"),
            "dbg_vn": nc.dram_tensor("dbg_vn", [128, 4 * (HEAD_DIM + 1)], f16, kind="ExternalOutput"),
            "dbg_cq0": nc.dram_tensor("dbg_cq0", [128, CH], f16, kind="ExternalOutput"),
            "dbg_cq1": nc.dram_tensor("dbg_cq1", [128, CH], f16, kind="ExternalOutput"),
            "dbg_rb": nc.dram_tensor("dbg_rb", [128, 2 * CH], f32, kind="ExternalOutput"),
            "dbg_rs": nc.dram_tensor("dbg_rs", [128, 2 * CH], f32, kind="ExternalOutput"),
            "dbg_ex": nc.dram_tensor("dbg_ex", [128, 2 * CH], f16, kind="ExternalOutput"),
            "dbg_sc": nc.dram_tensor("dbg_sc", [128, 2 * CH], f32, kind="ExternalOutput"),
        }

    with tile.TileContext(nc) as tc, ExitStack() as ctx:
        singles = ctx.enter_context(tc.tile_pool(name="singles", bufs=1))
        wq_sb = [singles.tile([128, 16, 128], f16, tag=f"wq{p}", name=f"wq{p}") for p in range(2)]
        wkv_sb = singles.tile([128, 16, 128], f16, tag="wkv", name="wkv")
        wo_sb = [singles.tile([128, D_OUT], f16, tag=f"wo{p}", name=f"wo{p}") for p in range(2)]
        ident = singles.tile([128, 128], f16, tag="ident", name="ident")

        # partition_broadcast needs the attn gpsimd ucode library (the
        # default `standard` library silently no-ops it on hardware)
        nc.gpsimd.load_library(library_config.attn)

        make_identity(nc, ident)

        # per-chunk static tiles (bufs = nch, requested once per chunk)
        kt_pool = ctx.enter_context(tc.tile_pool(name="kt", bufs=nch))
        vt_pool = ctx.enter_context(tc.tile_pool(name="vt", bufs=nch))
        v_pool = ctx.enter_context(tc.tile_pool(name="vn", bufs=nch))
        # rotating work tiles
        xt_pool = ctx.enter_context(tc.tile_pool(name="xt", bufs=2))
        qt_pool = ctx.enter_context(tc.tile_pool(name="qt", bufs=2))
        qs_pool = ctx.enter_context(tc.tile_pool(name="qs", bufs=3))
        ex_pool = ctx.enter_context(tc.tile_pool(name="ex", bufs=3))
        cq_pool = ctx.enter_context(tc.tile_pool(name="cq", bufs=4))
        tmp_pool = ctx.enter_context(tc.tile_pool(name="tmp", bufs=2))
        rs_pool = ctx.enter_context(tc.tile_pool(name="rs", bufs=2))
        rb_pool = ctx.enter_context(tc.tile_pool(name="rb", bufs=2))
        cs_pool = ctx.enter_context(tc.tile_pool(name="cs", bufs=2))
        osb_pool = ctx.enter_context(tc.tile_pool(name="osb", bufs=2))
        ps_sc = ctx.enter_context(tc.tile_pool(name="pssc", bufs=2, space="PSUM"))
        ps_ctx = ctx.enter_context(tc.tile_pool(name="psctx", bufs=1, space="PSUM"))
        ps_mix = ctx.enter_context(tc.tile_pool(name="psmix", bufs=2, space="PSUM"))

        xts, kts, vts, vns, qts, cqs = {}, {}, {}, {}, {}, {}
        dbg_rb_tile = []
        if DEBUG_DUMP:
            dbg_ex_sb = singles.tile([128, 2 * CH], f16, tag="dbgex", name="dbgex")
            dbg_sc_sb = singles.tile([128, 2 * CH], f32, tag="dbgsc", name="dbgsc")

        def dma_xt(c):
            t = xt_pool.tile([128, 16, CH], f16, tag="xt", name="xt")
            for h in range(2):
                nc.sync.dma_start(
                    out=t[:, ts(h, 8), :],
                    in_=xT[ts(h, 1024), ts(c, CH)].rearrange(
                        "(kc p) n -> p kc n", p=128
                    ),
                )
            xts[c] = t

        def proj_pieces(c):
            """q/kv projections + v transpose for chunk c -> list of thunks."""
            pieces = []
            st = {}

            def q_mms(p, k0):
                def f():
                    if k0 == 0:
                        st[f"pq{p}"] = ps_mix.tile([128, CH], f32, tag="mix", name="pq")
                    pq = st[f"pq{p}"]
                    for k in range(k0, k0 + 4):
                        nc.tensor.matmul(
                            pq, wq_sb[p][:, k, :], xts[c][:, k, :],
                            start=(k == 0), stop=(k == 15),
                        )
                return f

            def q_fix(p):
                def f():
                    pq = st[f"pq{p}"]
                    if p == 0:
                        qts[c] = qt_pool.tile([128, 4 * CH], f16, tag="qt", name="qt")
                    qt_t = qts[c]
                    nc.vector.tensor_copy(
                        out=qt_t[0:64, ts(2 * p, CH)], in_=pq[0:64, :]
                    )
                    qs_t = qs_pool.tile([128, CH], f16, tag="qs", name="qs")
                    nc.vector.tensor_copy(out=qs_t[64:128, :], in_=pq[64:128, :])
                    nc.sync.dma_start(
                        out=qt_t[0:64, ts(2 * p + 1, CH)], in_=qs_t[64:128, :]
                    )
                return f

            def kv_mms(k0):
                def f():
                    if k0 == 0:
                        st["pkv"] = ps_mix.tile([128, CH], f32, tag="mix", name="pkv")
                    pkv = st["pkv"]
                    for k in range(k0, k0 + 4):
                        nc.tensor.matmul(
                            pkv, wkv_sb[:, k, :], xts[c][:, k, :],
                            start=(k == 0), stop=(k == 15),
                        )
                return f

            def kv_fix():
                # psum rows 0:64 = vT chunk, rows 64:128 = kT chunk
                pkv = st["pkv"]
                vts[c] = vt_pool.tile([128, CH], f16, tag="vt", name="vt")
                nc.vector.tensor_copy(out=vts[c][0:64, :], in_=pkv[0:64, :])
                ks_t = qs_pool.tile([128, CH], f16, tag="qs", name="ks")
                nc.vector.tensor_copy(out=ks_t[64:128, :], in_=pkv[64:128, :])
                kts[c] = kt_pool.tile([128, CH], f16, tag="kt", name="kt")
                nc.sync.dma_start(out=kts[c][0:64, :], in_=ks_t[64:128, :])
                vns[c] = v_pool.tile([128, 4, HEAD_DIM + 1], f16, tag="vn", name="vn")
                nc.vector.memset(vns[c][:, :, HEAD_DIM : HEAD_DIM + 1], 1.0)

            def v_tr(j):
                def f():
                    pv = ps_mix.tile([128, CH], f16, tag="mix", name="pv")
                    nc.tensor.transpose(
                        pv[:, 0:HEAD_DIM],
                        vts[c][0:64, ts(j, BLK)],
                        ident[0:64, 0:64],
                    )
                    nc.vector.tensor_copy(
                        out=vns[c][:, j, 0:HEAD_DIM], in_=pv[:, 0:HEAD_DIM]
                    )
                return f

            # q pair 0 completes first (needs only wq0 + the xt halves in
            # arrival order), then pair 1
            pieces.append(q_mms(0, 0))
            pieces.append(q_mms(0, 4))
            pieces.append(q_mms(0, 8))
            pieces.append(q_mms(0, 12))
            pieces.append(q_fix(0))
            pieces.append(q_mms(1, 0))
            pieces.append(q_mms(1, 4))
            pieces.append(q_mms(1, 8))
            pieces.append(q_mms(1, 12))
            pieces.append(q_fix(1))
            for k0 in (0, 4, 8, 12):
                pieces.append(kv_mms(k0))
            pieces.append(kv_fix)
            for j in range(4):
                pieces.append(v_tr(j))
            return pieces

        def oproj_pieces(c):
            """out projection for chunk c (after normalize(c)) -> thunks."""
            pieces = []
            st = {}

            def mm(jj, g):
                def f():
                    if g == 0:
                        st[f"osb{jj}"] = osb_pool.tile(
                            [128, D_OUT], f16, tag="osb", name="osb"
                        )
                    po = [
                        ps_mix.tile([128, CH], f32, tag="mix", name="po")
                        for _ in range(2)
                    ]
                    for p in range(2):
                        for n in range(2):
                            nc.tensor.matmul(
                                po[n],
                                cqs[c][p][:, ts(jj, 128)],
                                wo_sb[p][:, ts(2 * g + n, CH)],
                                start=(p == 0), stop=(p == 1),
                            )
                    for n in range(2):
                        nc.vector.tensor_copy(
                            out=st[f"osb{jj}"][:, ts(2 * g + n, CH)], in_=po[n]
                        )
                return f

            def dma(jj):
                def f():
                    nc.sync.dma_start(
                        out=out[ts(4 * c + jj, BLK), :], in_=st[f"osb{jj}"]
                    )
                return f

            for jj in range(4):
                pieces.append(mm(jj, 0))
                pieces.append(mm(jj, 1))
                pieces.append(dma(jj))
            return pieces

        def do_pass(c, p, fillers):
            nblk = 4 * c + 4
            last = nblk - 1
            total = len(fillers)
            qt_t = qts[c]
            pctx = ps_ctx.tile([128, 2 * CH], f32, tag="ctx", name="pctx")
            for b in range(nblk):
                jd = b - 4 * c
                cc, j = b // 4, b % 4
                kt = kts[cc]
                sc = ps_sc.tile([128, 2 * CH], f32, tag="sc", name="sc")
                if jd < 0 and WIDE:
                    nc.tensor.matmul(
                        sc, kt[0:64, ts(j, BLK)], qt_t[0:64, ts(p, 2 * CH)],
                        start=True, stop=True,
                    )
                elif jd < 0:
                    for hh in range(2):
                        nc.tensor.matmul(
                            sc[:, ts(hh, CH)],
                            kt[0:64, ts(j, BLK)],
                            qt_t[0:64, ts(2 * p + hh, CH)],
                            start=True, stop=True,
                        )
                elif TRIM:
                    col0 = BLK * jd
                    for hh in range(2):
                        nc.tensor.matmul(
                            sc[:, hh * CH + col0 : (hh + 1) * CH],
                            kt[0:64, ts(j, BLK)],
                            qt_t[0:64, (2 * p + hh) * CH + col0 : (2 * p + hh + 1) * CH],
                            start=True, stop=True,
                        )
                else:
                    for hh in range(2):
                        nc.tensor.matmul(
                            sc[:, ts(hh, CH)],
                            kt[0:64, ts(j, BLK)],
                            qt_t[0:64, ts(2 * p + hh, CH)],
                            start=True, stop=True,
                        )
                ex = ex_pool.tile([128, 2 * CH], f16, tag="ex", name="ex")
                if jd < 0:
                    nc.scalar.activation(out=ex, in_=sc, func=Exp, scale=SCALE)
                elif TRIM:
                    col0 = BLK * jd
                    for hh in range(2):
                        nc.scalar.activation(
                            out=ex[:, hh * CH + col0 : (hh + 1) * CH],
                            in_=sc[:, hh * CH + col0 : (hh + 1) * CH],
                            func=Exp, scale=SCALE,
                        )
                    if jd > 0:
                        for hh in range(2):
                            nc.vector.memset(ex[:, hh * CH : hh * CH + col0], 0.0)
                    for hh in range(2):
                        # keep where q_local - r >= 0 within the 128-wide
                        # diagonal sub-block
                        nc.gpsimd.affine_select(
                            out=ex[:, hh * CH + col0 : hh * CH + col0 + BLK],
                            in_=ex[:, hh * CH + col0 : hh * CH + col0 + BLK],
                            compare_op=mybir.AluOpType.is_ge,
                            fill=0.0,
                            base=0,
                            pattern=[[1, BLK]],
                            channel_multiplier=-1,
                        )
                else:
                    nc.scalar.activation(out=ex, in_=sc, func=Exp, scale=SCALE)
                    for hh in range(2):
                        # baseline-style full-width diagonal mask:
                        # keep where q - r - 128*jd >= 0
                        nc.gpsimd.affine_select(
                            out=ex[:, ts(hh, CH)],
                            in_=ex[:, ts(hh, CH)],
                            compare_op=mybir.AluOpType.is_ge,
                            fill=0.0,
                            base=-BLK * jd,
                            pattern=[[1, CH]],
                            channel_multiplier=-1,
                        )
                if DEBUG_DUMP and c == 0 and p == 0 and b == 0:
                    nc.vector.tensor_copy(out=dbg_ex_sb, in_=ex)
                    nc.vector.tensor_copy(out=dbg_sc_sb, in_=sc)
                if WIDE:
                    nc.tensor.matmul(
                        pctx[: HEAD_DIM + 1, :], vns[cc][:, j, :], ex,
                        start=(b == 0), stop=(b == last),
                    )
                else:
                    for hh in range(2):
                        nc.tensor.matmul(
                            pctx[: HEAD_DIM + 1, ts(hh, CH)],
                            vns[cc][:, j, :], ex[:, ts(hh, CH)],
                            start=(b == 0), stop=(b == last),
                        )
                # drain fillers proportionally, starting from block 2
                if total and b >= 2:
                    want = total * (b - 1) // max(1, nblk - 2)
                    while len(fillers) > total - want:
                        fillers.popleft()()
            while fillers:
                fillers.popleft()()
            return pctx

        def normalize(c, p, pctx):
            # Stage ctx psum rows to SBUF immediately so the single psum ctx
            # tile frees fast (next pass's accumulation can start).
            # The custom DVE uop (reciprocal_approx_fast) and the gpsimd
            # ucode op (partition_broadcast) only work at partition base 0
            # on hardware, so DMA the denominator row down to partition 0.
            cs_t = cs_pool.tile([128, 2 * CH], f32, tag="cs", name="cs")
            nc.vector.tensor_copy(out=cs_t[0:65, :], in_=pctx[0:65, :])
            rs_t = rs_pool.tile([128, 2 * CH], f32, tag="rs", name="rs")
            nc.sync.dma_start(out=rs_t[0:1, :], in_=cs_t[64:65, :])
            rcp_t = rs_pool.tile([128, 2 * CH], f32, tag="rcp", name="rcp")
            nc.vector.reciprocal_approx_fast(
                out=rcp_t[0:1, :], in_=rs_t[0:1, :]
            )
            rb_t = rb_pool.tile([128, 2 * CH], f32, tag="rb", name="rb")
            nc.gpsimd.partition_broadcast(rb_t[0:64, :], rcp_t[0:1, :], channels=64)
            cq_t = cq_pool.tile([128, CH], f16, tag="cq", name="cq")
            tm_t = tmp_pool.tile([128, CH], f16, tag="tmp", name="tmp")
            nc.vector.tensor_mul(tm_t[0:64, :], cs_t[0:64, ts(1, CH)], rb_t[0:64, ts(1, CH)])
            nc.sync.dma_start(out=cq_t[64:128, :], in_=tm_t[0:64, :])
            nc.vector.tensor_mul(cq_t[0:64, :], cs_t[0:64, ts(0, CH)], rb_t[0:64, ts(0, CH)])
            cqs.setdefault(c, {})[p] = cq_t
            if DEBUG_DUMP and c == 0 and p == 0:
                dbg_rb_tile.append(rb_t)
                dbg_rb_tile.append(rs_t)

        # prologue: interleave weight and x-chunk loads so the first
        # projection matmuls (needing only wq0 + the first xT half) start
        # as early as possible; wo is not needed until oproj(0)
        xt0 = xt_pool.tile([128, 16, CH], f16, tag="xt", name="xt")
        xts[0] = xt0
        nc.sync.dma_start(out=wq_sb[0], in_=wq[0, :, :, :])
        for h in range(2):
            nc.sync.dma_start(
                out=xt0[:, ts(h, 8), :],
                in_=xT[ts(h, 1024), ts(0, CH)].rearrange(
                    "(kc p) n -> p kc n", p=128
                ),
            )
        nc.sync.dma_start(out=wq_sb[1], in_=wq[1, :, :, :])
        nc.sync.dma_start(out=wkv_sb, in_=wkv[:, :, :])
        if nch > 1:
            dma_xt(1)
        for p in range(2):
            nc.scalar.dma_start(out=wo_sb[p], in_=wo[p, :, :])
        for pc in proj_pieces(0):
            pc()

        pp_next = None
        for c in range(nch):
            if c >= 1 and c + 1 < nch:
                dma_xt(c + 1)
            if c == 0 and nch > 1:
                pp_next = proj_pieces(1)
                f0 = deque(pp_next[:5])
            else:
                f0 = deque(oproj_pieces(c - 1)) if c >= 1 else deque()
            if not FILL:
                while f0:
                    f0.popleft()()
            pctx0 = do_pass(c, 0, f0)
            normalize(c, 0, pctx0)
            if c == 0 and pp_next is not None:
                f1 = deque(pp_next[5:])
            else:
                f1 = deque(proj_pieces(c + 1)) if c + 1 < nch else deque()
            if not FILL:
                while f1:
                    f1.popleft()()
            pctx1 = do_pass(c, 1, f1)
            normalize(c, 1, pctx1)
        for pc in oproj_pieces(nch - 1):
            pc()
        if DEBUG_DUMP:
            nc.sync.dma_start(out=dbg["dbg_qt"][:, :], in_=qts[0])
            nc.sync.dma_start(out=dbg["dbg_kt"][:, :], in_=kts[0])
            nc.sync.dma_start(out=dbg["dbg_vt"][:, :], in_=vts[0])
            nc.sync.dma_start(out=dbg["dbg_vn"][:, :], in_=vns[0].rearrange("p a b -> p (a b)"))
            nc.sync.dma_start(out=dbg["dbg_cq0"][:, :], in_=cqs[0][0])
            nc.sync.dma_start(out=dbg["dbg_cq1"][:, :], in_=cqs[0][1])
            nc.sync.dma_start(out=dbg["dbg_rb"][:, :], in_=dbg_rb_tile[0])
            nc.sync.dma_start(out=dbg["dbg_rs"][:, :], in_=dbg_rb_tile[1])
            nc.sync.dma_start(out=dbg["dbg_ex"][:, :], in_=dbg_ex_sb)
            nc.sync.dma_start(out=dbg["dbg_sc"][:, :], in_=dbg_sc_sb)

    nc.compile()
    return nc


def _get_nc(s):
    if s not in _NC_CACHE:
        _NC_CACHE[s] = _build(s)
    return _NC_CACHE[s]


def _in_maps(x, Wq, Wk, Wv, Wo, s):
    xT = np.ascontiguousarray(np.asarray(x, np.float32)[0].T.astype(np.float16))
    Wq = np.asarray(Wq, np.float32).astype(np.float16)
    Wk = np.asarray(Wk, np.float32).astype(np.float16)
    Wv = np.asarray(Wv, np.float32).astype(np.float16)
    Wo = np.asarray(Wo, np.float32).astype(np.float16)
    maps = []
    for i in range(N_CORES):
        wq_i = np.ascontiguousarray(
            Wq[:, i * 256 : (i + 1) * 256]
            .reshape(16, 128, 2, 128)
            .transpose(2, 1, 0, 3)
        )
        wkv_i = np.ascontiguousarray(
            np.concatenate(
                [Wv[:, i * 64 : (i + 1) * 64], Wk[:, i * 64 : (i + 1) * 64]], axis=1
            )
            .reshape(16, 128, 128)
            .transpose(1, 0, 2)
        )
        wo_i = np.ascontiguousarray(
            Wo[i * 256 : (i + 1) * 256, :].reshape(2, 128, D_OUT)
        )
        maps.append({"xT": xT, "wq": wq_i, "wkv": wkv_i, "wo": wo_i})
    return maps


def run(x, Wq, Wk, Wv, Wo, bo, s=S, **spmd_kwargs):
    """Builds (cached), runs on 8 cores, returns (full_output, BassKernelResults)."""
    from concourse.bass_utils import run_bass_kernel_spmd

    nc = _get_nc(s)
    maps = _in_maps(x, Wq, Wk, Wv, Wo, s)
    res = run_bass_kernel_spmd(nc, maps, core_ids=list(range(N_CORES)), **spmd_kwargs)
    acc = np.zeros((s, D_OUT), np.float64)
    for r in res.results:
        acc += r["out"].astype(np.float64)
    full = (acc + np.asarray(bo, np.float64)[None, :]).astype(np.float32)[None]
    return full, res


def kernel(x, Wq, Wk, Wv, Wo, bo):
    out, _ = run(x, Wq, Wk, Wv, Wo, bo)
    return out
